# revision 38
# baseline (speedup 1.0000x reference)
"""Multi-head causal self-attention for TRN2, 8 NeuronCores.  ~285us
(baseline 405us).

Sharding: core i handles (batch b = i//2, head-group g = i%2); each head-group
is 8 of the 16 heads.  Per core everything is computed in "transposed" space
so no on-device transposes are needed; the host sums the two head-group
partial projections per batch and adds b_proj.

Design (evolved from the 405us baseline via perfetto-trace iteration):
  * Row-tiled QK: the two heads of a pair run as concurrent K=64 matmuls on
    PE row-groups (0,0)/(64,0) writing the two halves (separate PSUM banks)
    of one [128, 1024] score tile; both heads' S^T cost ~one 512-col stream.
  * One software-pipelined loop.  Phase-1 QKV projection for block j+1 and
    the output projection for block j-1 are diced into single-matmul
    "filler" items popped inside the exp-paced attention chunk loop by a
    static (est. PE-ns vs ACT-ns) scheduler with FIFO need()-gates, so the
    PE never idles long enough for the HAM clock gate to re-throttle
    (cold PE at 1.2 GHz instead of 2.4 was the baseline's main loss).
  * PV lags exp by 2 chunks so its semaphores are long-satisfied at issue.
  * Causal column skip: a diagonal chunk s streams only q-columns >= s*128
    in QK and PV, and the exp output is masked only in the 128-wide
    diagonal band (one [128,128] tri tile, two small DVE multiplies).
  * ACT does only exp (plus warm-table load): PSUM drains are DVE copies,
    QKV biases (zero here) would be rank-1 f32r matmuls.
  * Softmax denominators ride the PV matmuls as a ones-column in the staged
    V ([V_h|1] per head, tiles pre-memset to 1.0); normalization is
    reciprocal_approx_fast (DVE) + partition_broadcast (GPSIMD) + one DVE
    multiply, deferred into the filler queue so the PE stream never pauses.
  * All inputs arrive bf16 (host-converted, bit-identical to a device
    cast) in DMA-friendly per-tile layouts: one dma_start each for x-block
    (8 KB/partition lines), wv, wproj, and per-m wqk tiles ordered so the
    first attention chunk's producers land first; output is written bf16.
  * 26 full-array junk matmuls at t=0 warm the HAM clock gate during the
    initial DMA wait (small matmuls don't register as PE activity).
  * The last block's projection is emitted pair-major into 8 parallel PSUM
    accumulators so the tail only waits on the final pair's normalization,
    and the first two blocks' projections are deferred into the last
    attention block, which otherwise runs out of filler and lets the PE
    cool down.
"""

import numpy as np
import ml_dtypes
from collections import deque
from contextlib import ExitStack

import concourse.bass as bass
import concourse.mybir as mybir
import concourse.tile as tile
from concourse import bacc
from concourse.bass_utils import run_bass_kernel_spmd

B, T, D, H = 4, 2048, 1024, 16
DK = 64            # head dim
HL = 8             # heads per core
DL = HL * DK       # 512 local head dims per core
N_CORES = 8

F32 = mybir.dt.float32
F32R = mybir.dt.float32r
BF16 = mybir.dt.bfloat16
EXP = mybir.ActivationFunctionType.Exp

TQ = 512           # tq block size
TKC = 128          # tk chunk size
NQB = T // TQ      # 4
NKC = T // TKC     # 16
NDCH = D // 128    # 8 contraction chunks over D
VSW = HL * 65 + 64  # staged-V width: 8*[V_h|1] + ones tail pad for M=128 lhsT

# rough per-instruction engine-busy estimates (ns) for the static scheduler
EST_QK = 320       # two concurrent row-tiled K=64 matmuls, N=512
EST_PV = 450       # two K=128 matmuls, N=512
EST_EXP = 1050     # ACT exp on [128, 1024] (measured)
EST_FILL = 230     # one N=512 matmul
RESERVE = 300

_CACHE = {}


def _build(causal: bool, qkv_bias: bool):
    nc = bacc.Bacc("TRN2", target_bir_lowering=False, debug=False,
                   num_devices=N_CORES)
    xT_d = nc.dram_tensor("xT", [NQB, 128, NDCH * TQ], BF16,
                          kind="ExternalInput").ap()
    wqk_d = nc.dram_tensor("wqk", [8, 128, 2 * DL], BF16, kind="ExternalInput").ap()
    wv_d = nc.dram_tensor("wv", [128, NDCH * DL], BF16,
                          kind="ExternalInput").ap()
    wp_d = nc.dram_tensor("wproj", [128, 4 * D], BF16,
                          kind="ExternalInput").ap()
    bqk_d = nc.dram_tensor("bqk", [1, 2 * DL], F32, kind="ExternalInput").ap()
    bv_d = nc.dram_tensor("bv", [1, DL], F32, kind="ExternalInput").ap()
    masks_d = nc.dram_tensor("masks", [TKC, TKC], BF16,
                             kind="ExternalInput").ap()
    out_d = nc.dram_tensor("out", [T, D], BF16, kind="ExternalOutput").ap()

    with tile.TileContext(nc) as tc, ExitStack() as top:
        persist = top.enter_context(tc.tile_pool(name="persist", bufs=1))
        xrpool = top.enter_context(
            tc.tile_pool(name="xrpool", bufs=2 if causal else 4))
        ps_s = top.enter_context(tc.tile_pool(name="ps_s", bufs=2, space="PSUM"))
        ps_o = top.enter_context(tc.tile_pool(name="ps_o", bufs=2, space="PSUM"))
        ps_sh = top.enter_context(tc.tile_pool(name="ps_sh", bufs=2, space="PSUM"))
        ppool = top.enter_context(tc.tile_pool(name="ppool", bufs=8))
        npool = top.enter_context(tc.tile_pool(name="npool", bufs=4))
        opool = top.enter_context(tc.tile_pool(name="opool", bufs=3))

        # ---------------- persistent tiles ----------------
        q2 = [persist.tile([128, T], BF16, tag=f"q2{i}", name=f"q2{i}")
              for i in range(4)]       # head-pair packed Q^T
        kT = [persist.tile([128, T], BF16, tag=f"kT{i}", name=f"kT{i}")
              for i in range(4)]       # head-pair packed K^T
        vs = [persist.tile([128, VSW], BF16, tag=f"vs{t}", name=f"vs{t}")
              for t in range(NKC)]     # staged V: [V_h|1]*8 + ones tail
        yT = [persist.tile([128, T], BF16, tag=f"yT{i}", name=f"yT{i}")
              for i in range(4)]
        wqk_m = [persist.tile([128, 2 * DL], BF16, tag=f"wqkm{m}",
                              name=f"wqkm{m}")
                 for m in range(8)]
        wv2 = persist.tile([128, NDCH * DL], BF16, tag="wv2", name="wv2")
        wp2 = persist.tile([128, 4 * D], BF16, tag="wp2", name="wp2")
        ones_r = persist.tile([1, 128], F32R, tag="ones_r", name="ones_r")
        maskb = None
        if causal:
            maskb = persist.tile([TKC, TKC], BF16, tag="maskb", name="maskb")
            nc.gpsimd.dma_start(maskb[:], masks_d)

        # ---------------- preamble ----------------
        initp = top.enter_context(tc.tile_pool(name="initp", bufs=1))
        ones_f = initp.tile([1, 512], F32, tag="ones_f", name="ones_f")
        nc.vector.memset(ones_f[:], 1.0)
        nc.vector.tensor_copy(ones_r[:], ones_f[:, 0:128])
        bqk_r = bv_r = ones512_r = None
        if qkv_bias:
            ones512_r = initp.tile([1, 512], F32R, tag="ones512_r",
                                   name="ones512_r")
            nc.vector.tensor_copy(ones512_r[:], ones_f[:])
            bqk_f = initp.tile([1, 2 * DL], F32, tag="bqk_f", name="bqk_f")
            nc.gpsimd.dma_start(bqk_f[:], bqk_d)
            bqk_r = initp.tile([1, 2 * DL], F32R, tag="bqk_r", name="bqk_r")
            nc.vector.tensor_copy(bqk_r[:], bqk_f[:])
            bv_f = initp.tile([1, DL], F32, tag="bv_f", name="bv_f")
            nc.gpsimd.dma_start(bv_f[:], bv_d)
            bv_r = initp.tile([1, DL], F32R, tag="bv_r", name="bv_r")
            nc.vector.tensor_copy(bv_r[:], bv_f[:])

        # x block 0 first, split across two queues so it lands early
        xr0 = xrpool.tile([128, NDCH * TQ], BF16, tag="xr", name="xr0")
        half = NDCH * TQ // 2
        nc.sync.dma_start(xr0[:, 0:half], xT_d[0][:, 0:half])
        nc.gpsimd.dma_start(xr0[:, half:], xT_d[0][:, half:])

        # weights: direct bf16 DMA into resident tiles
        # queue order = arrival order: attention's first needs go first
        nc.scalar.dma_start(wqk_m[0][:], wqk_d[0])
        nc.scalar.dma_start(wqk_m[4][:], wqk_d[4])
        nc.scalar.dma_start(wv2[:], wv_d)
        for m in (1, 2, 3):
            nc.gpsimd.dma_start(wqk_m[m][:], wqk_d[m])
        for m in (5, 6, 7):
            nc.scalar.dma_start(wqk_m[m][:], wqk_d[m])
        nc.scalar.dma_start(wp2[:], wp_d)

        # staged-V tiles start as all-ones; the V copies overwrite the V
        # columns and leave the |1 columns and the tail as ones.
        for t in range(NKC):
            nc.vector.memset(vs[t][:], 1.0)

        # PE warm-up: full-array junk matmuls during the initial DMA wait
        # so the HAM clock gate reaches K=8/8 before phase-1 work arrives
        # (small matmuls don't register as "busy" in the activity monitor).
        wup = ps_sh.tile([128, TQ], F32, tag="sh", name="warmup_ps")
        for _ in range(20):
            nc.tensor.matmul(wup[:], vs[0][:, 0:128], vs[1][:, 0:TQ],
                             start=True, stop=True)

        # ---------------- x loads (DMA + gpsimd cast) ----------------
        xr_cache = {0: xr0}

        def xload(j):
            if j in xr_cache:
                return
            xr_j = xrpool.tile([128, NDCH * TQ], BF16, tag="xr",
                               name=f"xr{j}")
            nc.sync.dma_start(xr_j[:], xT_d[j])
            xr_cache[j] = xr_j

        # ---------------- filler machinery ----------------
        filler = deque()   # items: (label_or_None, fn, est_pe_ns)
        done = set()
        est = {"pe": 0.0, "act": 0.0}

        def pop_one():
            label, fn, cost = filler.popleft()
            fn()
            if label is not None:
                done.add(label)
            est["pe"] += cost

        def need(label):
            while label not in done:
                assert filler, f"gate {label} not in filler"
                pop_one()

        def budget_pops():
            while filler and est["pe"] + RESERVE < est["act"]:
                pop_one()

        def ph1_steps(j):
            """Phase-1 QKV projection for query block j as filler items."""
            jsl = slice(j * TQ, (j + 1) * TQ)
            xr_j = xr_cache[j]
            items = []

            def m_group(m):
                cell = {}

                def mk(d):
                    def fn():
                        if d == 0:
                            cell["ps"] = ps_sh.tile(
                                [128, TQ], F32, tag="sh", name=f"psqk{j}_{m}")
                        ps = cell["ps"]
                        last = (d == NDCH - 1) and not qkv_bias
                        nc.tensor.matmul(
                            ps[:], wqk_m[m][:, d * 128:(d + 1) * 128],
                            xr_j[:, d * TQ:(d + 1) * TQ],
                            start=(d == 0), stop=last)
                        if d == NDCH - 1:
                            if qkv_bias:
                                nc.tensor.matmul(
                                    ps[:], bqk_r[0:1, m * 128:(m + 1) * 128],
                                    ones512_r[:], start=False, stop=True)
                            dst = q2[m] if m < 4 else kT[m - 4]
                            nc.vector.tensor_copy(dst[:, jsl], ps[:])
                    return fn

                return ([(None, mk(d), EST_FILL) for d in range(NDCH - 1)]
                        + [(("m", j, m), mk(NDCH - 1), EST_FILL + 60)])

            def v_group(tt):
                c = tt % 4
                cell = {}

                def mk(d):
                    def fn():
                        if d == 0:
                            cell["ps"] = ps_sh.tile(
                                [128, DL], F32, tag="sh", name=f"psv{tt}")
                        ps = cell["ps"]
                        last = (d == NDCH - 1) and not qkv_bias
                        nc.tensor.matmul(
                            ps[:],
                            xr_j[:, d * TQ + c * 128:d * TQ + (c + 1) * 128],
                            wv2[:, d * DL:(d + 1) * DL],
                            start=(d == 0), stop=last)
                        if d == NDCH - 1:
                            if qkv_bias:
                                nc.tensor.matmul(
                                    ps[:], ones_r[:, 0:128], bv_r[:],
                                    start=False, stop=True)
                            src = ps.rearrange("p (h x) -> p h x", h=HL)
                            dst = vs[tt][:, 0:HL * 65].rearrange(
                                "p (h x) -> p h x", x=65)[:, :, 0:64]
                            nc.vector.tensor_copy(dst, src)
                    return fn

                return ([(None, mk(d), EST_FILL) for d in range(NDCH - 1)]
                        + [(("v", j, tt % 4), mk(NDCH - 1), EST_FILL + 60)])

            # pair-i Q/K groups unlock attention pairs in sequence; V groups
            # interleave so forced drains stay small.
            for i in range(4):
                items += m_group(i) + m_group(4 + i)
                items += v_group(4 * j + i)
            return items

        def proj_steps(j):
            items = []
            for t in range(4 * j, 4 * j + 4):
                for nb in range(2):
                    def fn(t=t, nb=nb):
                        nsl = slice(nb * 512, (nb + 1) * 512)
                        ps3 = ps_sh.tile([128, TQ], F32, tag="sh",
                                         name=f"ps3_{t}_{nb}")
                        for k in range(4):
                            nc.tensor.matmul(
                                ps3[:], yT[k][:, t * 128:(t + 1) * 128],
                                wp2[:, k * D + nb * 512:k * D + (nb + 1) * 512],
                                start=(k == 0), stop=(k == 3))
                        ot = opool.tile([128, TQ], BF16, tag="ot",
                                        name=f"ot{t}_{nb}")
                        nc.vector.tensor_copy(ot[:], ps3[:])
                        outq[(t + nb) % 2].dma_start(
                            out_d[t * 128:(t + 1) * 128, nsl], ot[:])
                    items.append((None, fn, 4 * EST_FILL))
            return items

        outq = [nc.gpsimd, nc.scalar]
        pair_no = [0]      # global head-pair counter (norm gating)
        proj_hold = []     # early blocks' projection, deferred to the last
                           # attention block where the PE would otherwise
                           # run out of filler and go cold

        # ---------------- main pipelined loop ----------------
        if causal:
            xload(1)
            filler.extend(ph1_steps(0))
        else:
            for j in range(1, NQB):
                xload(j)
            for j in range(NQB):
                filler.extend(ph1_steps(j))

        for j in range(NQB):
            jsl = slice(j * TQ, (j + 1) * TQ)
            cs = list(range(4 * (j + 1))) if causal else list(range(NKC))
            if causal and j + 2 < NQB:
                xload(j + 2)
            if causal and j + 1 < NQB:
                filler.extend(ph1_steps(j + 1))
            if causal and j == NQB - 1:
                filler.extend(proj_hold)
                proj_hold.clear()

            for i in range(4):          # head pair (2i, 2i+1)
                hA, hB = 2 * i, 2 * i + 1
                need(("m", j, i))
                poA = ps_o.tile([128, TQ], F32, tag="po", name=f"poA{j}_{i}")
                poB = ps_o.tile([128, TQ], F32, tag="po", name=f"poB{j}_{i}")

                pend = deque()  # pipeline: PV(c) emitted after QK(c+2)
                first_pv = [True]

                def pv_pop(stop):
                    pc, ppt = pend.popleft()
                    need(("v", pc // 4, pc % 4))
                    pv_emit(pc, ppt, first_pv[0], stop)
                    first_pv[0] = False
                def qskip(c):
                    # fully-masked leading query columns of a diagonal chunk
                    if causal and c >= 4 * j:
                        return (c - 4 * j) * TKC
                    return 0

                def pv_emit(pc, ppt, start, stop):
                    k0 = qskip(pc)
                    nc.tensor.matmul(
                        poA[:, k0:TQ], vs[pc][:, hA * 65:hA * 65 + 128],
                        ppt[:, k0:TQ], start=start, stop=stop)
                    nc.tensor.matmul(
                        poB[:, k0:TQ], vs[pc][:, hB * 65:hB * 65 + 128],
                        ppt[:, TQ + k0:2 * TQ], start=start, stop=stop)
                    est["pe"] += EST_PV

                for ci, c in enumerate(cs):
                    need(("m", c // 4, 4 + i))
                    csl = slice(c * TKC, (c + 1) * TKC)
                    k0 = qskip(c)
                    ss = ps_s.tile([TKC, 2 * TQ], F32, tag="ss",
                                   name=f"ss{j}_{i}_{c}")
                    nc.tensor.matmul(ss[:, k0:TQ], kT[i][0:64, csl],
                                     q2[i][0:64, j * TQ + k0:(j + 1) * TQ],
                                     start=True, stop=True)
                    nc.tensor.matmul(ss[:, TQ + k0:2 * TQ], kT[i][64:128, csl],
                                     q2[i][64:128, j * TQ + k0:(j + 1) * TQ],
                                     start=True, stop=True)
                    est["pe"] += EST_QK
                    pt = ppool.tile([TKC, 2 * TQ], BF16, tag="pt",
                                    name=f"pt{j}_{i}_{c}")
                    nc.scalar.activation(pt[:], ss[:], EXP, scale=0.125)
                    est["act"] += EST_EXP
                    if causal and c >= 4 * j:
                        # only the 128-wide diagonal band is partially masked
                        bsl = slice(k0, k0 + TKC)
                        nc.vector.tensor_mul(pt[:, bsl], pt[:, bsl], maskb[:])
                        bslB = slice(TQ + k0, TQ + k0 + TKC)
                        nc.vector.tensor_mul(pt[:, bslB], pt[:, bslB],
                                             maskb[:])
                    if len(pend) >= 3:
                        pv_pop(False)
                    pend.append((c, pt))
                    # bound accounting drift so filler keeps flowing through
                    # DMA-paced stretches where the PE is stall-bound anyway
                    if est["pe"] > est["act"] + 3000:
                        est["pe"] = est["act"] + 3000
                    budget_pops()
                while len(pend) > 1:
                    pv_pop(False)
                budget_pops()
                pv_pop(True)

                # immediate DVE part of softmax normalization; the
                # broadcast+multiply is queued as a filler item so the PE
                # stream never pauses at pair/phase boundaries.
                if pair_no[0] >= 2:
                    need(("n", pair_no[0] - 2))   # npool buffer rotation
                rr, osb = [], []
                for h, po in ((hA, poA), (hB, poB)):
                    o_sb = npool.tile([64, TQ], BF16, tag="o_sb",
                                      name=f"ob{j}_{h}")
                    nc.vector.tensor_copy(o_sb[:], po[0:64, :])
                    sums = npool.tile([1, TQ], F32, tag="sums",
                                      name=f"sm{j}_{h}")
                    nc.vector.tensor_copy(sums[:], po[64:65, :])
                    recip = npool.tile([1, TQ], F32, tag="recip",
                                       name=f"rc{j}_{h}")
                    nc.vector.reciprocal_approx_fast(
                        out=recip[:], in_=sums[:])
                    rr.append(recip)
                    osb.append(o_sb)

                def norm_fn(i=i, jsl=jsl, osb=osb, rr=rr, j=j, hA=hA):
                    for hp in (0, 1):
                        pb = npool.tile([64, TQ], F32, tag="pb",
                                        name=f"pb{j}_{hA + hp}")
                        nc.gpsimd.partition_broadcast(pb[:], rr[hp][:])
                        nc.vector.tensor_mul(
                            yT[i][hp * 64:(hp + 1) * 64, jsl],
                            osb[hp][:], pb[:])
                filler.append((("n", pair_no[0]), norm_fn, 50))
                pair_no[0] += 1

            if causal and j == NQB - 1:
                break                  # final block projection done below
            if causal and j < 2:
                proj_hold.extend(proj_steps(j))
            else:
                filler.extend(proj_steps(j))

        if causal:
            while filler:              # drain ph1/proj/norm backlog
                pop_one()
            # final-block projection, pair-major: MMs for pair k only wait
            # norm(k), so the PE streams through the tail without stalls.
            jf = NQB - 1
            s1 = ps_s.tile([TKC, 2 * TQ], F32, tag="ss", name="pf_a")
            s2 = ps_s.tile([TKC, 2 * TQ], F32, tag="ss", name="pf_b")
            s3 = ps_sh.tile([128, TQ], F32, tag="sh", name="pf_c")
            s4 = ps_sh.tile([128, TQ], F32, tag="sh", name="pf_d")
            s5 = ps_o.tile([128, TQ], F32, tag="po", name="pf_e")
            s6 = ps_o.tile([128, TQ], F32, tag="po", name="pf_f")
            slots = [s1[:, 0:TQ], s1[:, TQ:2 * TQ], s2[:, 0:TQ],
                     s2[:, TQ:2 * TQ], s3[:], s4[:], s5[:], s6[:]]
            steps = [(t, nb) for t in range(4 * jf, 4 * jf + 4)
                     for nb in range(2)]
            for k in range(4):
                for idx, (t, nb) in enumerate(steps):
                    nsl = slice(nb * 512, (nb + 1) * 512)
                    nc.tensor.matmul(
                        slots[idx], yT[k][:, t * 128:(t + 1) * 128],
                        wp2[:, k * D + nb * 512:k * D + (nb + 1) * 512],
                        start=(k == 0), stop=(k == 3))
                    if k == 3:
                        ot = opool.tile([128, TQ], BF16, tag="ot",
                                        name=f"otf{idx}")
                        nc.vector.tensor_copy(ot[:], slots[idx])
                        outq[idx % 2].dma_start(
                            out_d[t * 128:(t + 1) * 128, nsl], ot[:])

        while filler:                  # flush remaining projection work
            pop_one()

    nc.compile()
    return nc


def _get_nc(causal: bool, qkv_bias: bool = False):
    key = (causal, qkv_bias)
    if key not in _CACHE:
        _CACHE[key] = _build(causal, qkv_bias)
    return _CACHE[key]


def _host_masks() -> np.ndarray:
    i = np.arange(TKC)[:, None]
    jj = np.arange(TKC)[None, :]
    return np.ascontiguousarray(
        (jj >= i).astype(np.float32).astype(ml_dtypes.bfloat16))


def _make_in_maps(x, W_qkv, b_qkv, W_proj):
    masks_np = _host_masks()
    in_maps = []
    for core in range(N_CORES):
        b, g = core // 2, core % 2
        qc = slice(g * DL, (g + 1) * DL)
        kc = slice(D + g * DL, D + (g + 1) * DL)
        vc = slice(2 * D + g * DL, 2 * D + (g + 1) * DL)
        bf = ml_dtypes.bfloat16
        wqk_full = np.concatenate([W_qkv[:, qc], W_qkv[:, kc]], axis=1)
        # [D, 2DL] -> [m, p, d, m2] so each m-chunk is one contiguous DMA
        wqk_t = wqk_full.reshape(8, 128, 8, 128).transpose(2, 1, 0, 3)
        in_maps.append({
            "xT": np.ascontiguousarray(
                x[b].T.reshape(NDCH, 128, NQB, TQ).transpose(2, 1, 0, 3)
                .reshape(NQB, 128, NDCH * TQ).astype(bf)),
            "wqk": np.ascontiguousarray(
                wqk_t.reshape(8, 128, 2 * DL).astype(bf)),
            "wv": np.ascontiguousarray(
                W_qkv[:, vc].reshape(NDCH, 128, DL).transpose(1, 0, 2)
                .reshape(128, NDCH * DL).astype(bf)),
            "bqk": np.ascontiguousarray(
                np.concatenate([b_qkv[qc], b_qkv[kc]]).reshape(1, 2 * DL)),
            "bv": np.ascontiguousarray(b_qkv[vc].reshape(1, DL)),
            "wproj": np.ascontiguousarray(
                W_proj[g * DL:(g + 1) * DL, :].reshape(4, 128, D)
                .transpose(1, 0, 2).reshape(128, 4 * D).astype(bf)),
            "masks": masks_np,
        })
    return in_maps


def kernel(x, mask, W_qkv, b_qkv, W_proj, b_proj):
    x = np.asarray(x, dtype=np.float32)
    mask2d = np.asarray(mask, dtype=np.int32).reshape(T, T)
    W_qkv = np.asarray(W_qkv, dtype=np.float32)
    b_qkv = np.asarray(b_qkv, dtype=np.float32)
    W_proj = np.asarray(W_proj, dtype=np.float32)
    b_proj = np.asarray(b_proj, dtype=np.float32)

    if np.array_equal(mask2d, np.tril(np.ones((T, T), dtype=np.int32))):
        causal = True
    elif np.all(mask2d == 1):
        causal = False
    else:
        raise NotImplementedError("only causal (tril) or all-ones masks")

    qkv_bias = bool(np.any(b_qkv != 0.0))
    nc = _get_nc(causal, qkv_bias)
    in_maps = _make_in_maps(x, W_qkv, b_qkv, W_proj)
    res = run_bass_kernel_spmd(nc, in_maps, core_ids=list(range(N_CORES)))
    out = np.empty((B, T, D), dtype=np.float32)
    for b in range(B):
        out[b] = (res.results[2 * b]["out"].astype(np.float32)
                  + res.results[2 * b + 1]["out"].astype(np.float32)
                  + b_proj[None, :])
    return out


# revision 39
# speedup vs baseline: 1.0079x; 1.0079x over previous
"""Multi-head causal self-attention for TRN2, 8 NeuronCores.  ~285us
(baseline 405us).

Sharding: core i handles (batch b = i//2, head-group g = i%2); each head-group
is 8 of the 16 heads.  Per core everything is computed in "transposed" space
so no on-device transposes are needed; the host sums the two head-group
partial projections per batch and adds b_proj.

Design (evolved from the 405us baseline via perfetto-trace iteration):
  * Row-tiled QK: the two heads of a pair run as concurrent K=64 matmuls on
    PE row-groups (0,0)/(64,0) writing the two halves (separate PSUM banks)
    of one [128, 1024] score tile; both heads' S^T cost ~one 512-col stream.
  * One software-pipelined loop.  Phase-1 QKV projection for block j+1 and
    the output projection for block j-1 are diced into single-matmul
    "filler" items popped inside the exp-paced attention chunk loop by a
    static (est. PE-ns vs ACT-ns) scheduler with FIFO need()-gates, so the
    PE never idles long enough for the HAM clock gate to re-throttle
    (cold PE at 1.2 GHz instead of 2.4 was the baseline's main loss).
  * PV lags exp by 2 chunks so its semaphores are long-satisfied at issue.
  * Causal column skip: a diagonal chunk s streams only q-columns >= s*128
    in QK and PV, and the exp output is masked only in the 128-wide
    diagonal band (one [128,128] tri tile, two small DVE multiplies).
  * ACT does only exp (plus warm-table load): PSUM drains are DVE copies,
    QKV biases (zero here) would be rank-1 f32r matmuls.
  * Softmax denominators ride the PV matmuls as a ones-column in the staged
    V ([V_h|1] per head, tiles pre-memset to 1.0); normalization is
    reciprocal_approx_fast (DVE) + partition_broadcast (GPSIMD) + one DVE
    multiply, deferred into the filler queue so the PE stream never pauses.
  * All inputs arrive bf16 (host-converted, bit-identical to a device
    cast) in DMA-friendly per-tile layouts: one dma_start each for x-block
    (8 KB/partition lines), wv, wproj, and per-m wqk tiles ordered so the
    first attention chunk's producers land first; output is written bf16.
  * 20 full-array junk matmuls at t=0 warm the HAM clock gate during the
    initial DMA wait (small matmuls don't register as PE activity).
  * The last block's projection is emitted pair-major into 8 parallel PSUM
    accumulators so the tail only waits on the final pair's normalization,
    and the first two blocks' projections are deferred into the last
    attention block, which otherwise runs out of filler and lets the PE
    cool down.
"""

import numpy as np
import ml_dtypes
from collections import deque
from contextlib import ExitStack

import concourse.bass as bass
import concourse.mybir as mybir
import concourse.tile as tile
from concourse import bacc
from concourse.bass_utils import run_bass_kernel_spmd

B, T, D, H = 4, 2048, 1024, 16
DK = 64            # head dim
HL = 8             # heads per core
DL = HL * DK       # 512 local head dims per core
N_CORES = 8

F32 = mybir.dt.float32
F32R = mybir.dt.float32r
BF16 = mybir.dt.bfloat16
EXP = mybir.ActivationFunctionType.Exp

TQ = 512           # tq block size
TKC = 128          # tk chunk size
NQB = T // TQ      # 4
NKC = T // TKC     # 16
NDCH = D // 128    # 8 contraction chunks over D
VSW = HL * 65 + 64  # staged-V width: 8*[V_h|1] + ones tail pad for M=128 lhsT

# rough per-instruction engine-busy estimates (ns) for the static scheduler
EST_QK = 320       # two concurrent row-tiled K=64 matmuls, N=512
EST_PV = 450       # two K=128 matmuls, N=512
EST_EXP = 1050     # ACT exp on [128, 1024] (measured)
EST_FILL = 230     # one N=512 matmul
RESERVE = 300

_CACHE = {}


def _build(causal: bool, qkv_bias: bool):
    nc = bacc.Bacc("TRN2", target_bir_lowering=False, debug=False,
                   num_devices=N_CORES)
    xT_d = nc.dram_tensor("xT", [NQB, 128, NDCH * TQ], BF16,
                          kind="ExternalInput").ap()
    wqk_d = nc.dram_tensor("wqk", [8, 128, 2 * DL], BF16, kind="ExternalInput").ap()
    wv_d = nc.dram_tensor("wv", [128, NDCH * DL], BF16,
                          kind="ExternalInput").ap()
    wp_d = nc.dram_tensor("wproj", [128, 4 * D], BF16,
                          kind="ExternalInput").ap()
    bqk_d = nc.dram_tensor("bqk", [1, 2 * DL], F32, kind="ExternalInput").ap()
    bv_d = nc.dram_tensor("bv", [1, DL], F32, kind="ExternalInput").ap()
    masks_d = nc.dram_tensor("masks", [TKC, TKC], BF16,
                             kind="ExternalInput").ap()
    out_d = nc.dram_tensor("out", [T, D], BF16, kind="ExternalOutput").ap()

    with tile.TileContext(nc) as tc, ExitStack() as top:
        persist = top.enter_context(tc.tile_pool(name="persist", bufs=1))
        xrpool = top.enter_context(
            tc.tile_pool(name="xrpool", bufs=2 if causal else 4))
        ps_s = top.enter_context(tc.tile_pool(name="ps_s", bufs=2, space="PSUM"))
        ps_o = top.enter_context(tc.tile_pool(name="ps_o", bufs=2, space="PSUM"))
        ps_sh = top.enter_context(tc.tile_pool(name="ps_sh", bufs=2, space="PSUM"))
        ppool = top.enter_context(tc.tile_pool(name="ppool", bufs=8))
        npool = top.enter_context(tc.tile_pool(name="npool", bufs=4))
        opool = top.enter_context(tc.tile_pool(name="opool", bufs=3))

        # ---------------- persistent tiles ----------------
        q2 = [persist.tile([128, T], BF16, tag=f"q2{i}", name=f"q2{i}")
              for i in range(4)]       # head-pair packed Q^T
        kT = [persist.tile([128, T], BF16, tag=f"kT{i}", name=f"kT{i}")
              for i in range(4)]       # head-pair packed K^T
        vs = [persist.tile([128, VSW], BF16, tag=f"vs{t}", name=f"vs{t}")
              for t in range(NKC)]     # staged V: [V_h|1]*8 + ones tail
        yT = [persist.tile([128, T], BF16, tag=f"yT{i}", name=f"yT{i}")
              for i in range(4)]
        wqk_m = [persist.tile([128, 2 * DL], BF16, tag=f"wqkm{m}",
                              name=f"wqkm{m}")
                 for m in range(8)]
        wv2 = persist.tile([128, NDCH * DL], BF16, tag="wv2", name="wv2")
        wp2 = persist.tile([128, 4 * D], BF16, tag="wp2", name="wp2")
        ones_r = persist.tile([1, 128], F32R, tag="ones_r", name="ones_r")
        maskb = None
        if causal:
            maskb = persist.tile([TKC, TKC], BF16, tag="maskb", name="maskb")
            nc.gpsimd.dma_start(maskb[:], masks_d)

        # ---------------- preamble ----------------
        initp = top.enter_context(tc.tile_pool(name="initp", bufs=1))
        ones_f = initp.tile([1, 512], F32, tag="ones_f", name="ones_f")
        nc.vector.memset(ones_f[:], 1.0)
        nc.vector.tensor_copy(ones_r[:], ones_f[:, 0:128])
        bqk_r = bv_r = ones512_r = None
        if qkv_bias:
            ones512_r = initp.tile([1, 512], F32R, tag="ones512_r",
                                   name="ones512_r")
            nc.vector.tensor_copy(ones512_r[:], ones_f[:])
            bqk_f = initp.tile([1, 2 * DL], F32, tag="bqk_f", name="bqk_f")
            nc.gpsimd.dma_start(bqk_f[:], bqk_d)
            bqk_r = initp.tile([1, 2 * DL], F32R, tag="bqk_r", name="bqk_r")
            nc.vector.tensor_copy(bqk_r[:], bqk_f[:])
            bv_f = initp.tile([1, DL], F32, tag="bv_f", name="bv_f")
            nc.gpsimd.dma_start(bv_f[:], bv_d)
            bv_r = initp.tile([1, DL], F32R, tag="bv_r", name="bv_r")
            nc.vector.tensor_copy(bv_r[:], bv_f[:])

        # x block 0 first, split across two queues so it lands early
        xr0 = xrpool.tile([128, NDCH * TQ], BF16, tag="xr", name="xr0")
        half = NDCH * TQ // 2
        nc.sync.dma_start(xr0[:, 0:half], xT_d[0][:, 0:half])
        nc.gpsimd.dma_start(xr0[:, half:], xT_d[0][:, half:])

        # weights: direct bf16 DMA into resident tiles
        # queue order = arrival order: attention's first needs go first
        nc.scalar.dma_start(wqk_m[0][:], wqk_d[0])
        nc.scalar.dma_start(wqk_m[4][:], wqk_d[4])
        nc.scalar.dma_start(wv2[:], wv_d)
        for m in (1, 2, 3):
            nc.gpsimd.dma_start(wqk_m[m][:], wqk_d[m])
        for m in (5, 6, 7):
            nc.scalar.dma_start(wqk_m[m][:], wqk_d[m])
        nc.scalar.dma_start(wp2[:], wp_d)

        # staged-V tiles start as all-ones; the V copies overwrite the V
        # columns and leave the |1 columns and the tail as ones.
        for t in range(NKC):
            nc.vector.memset(vs[t][:], 1.0)

        # PE warm-up: full-array junk matmuls during the initial DMA wait
        # so the HAM clock gate reaches K=8/8 before phase-1 work arrives
        # (small matmuls don't register as "busy" in the activity monitor).
        wup = ps_sh.tile([128, TQ], F32, tag="sh", name="warmup_ps")
        for _ in range(20):
            nc.tensor.matmul(wup[:], vs[0][:, 0:128], vs[1][:, 0:TQ],
                             start=True, stop=True)

        # ---------------- x loads (DMA + gpsimd cast) ----------------
        xr_cache = {0: xr0}

        def xload(j):
            if j in xr_cache:
                return
            xr_j = xrpool.tile([128, NDCH * TQ], BF16, tag="xr",
                               name=f"xr{j}")
            nc.sync.dma_start(xr_j[:], xT_d[j])
            xr_cache[j] = xr_j

        # ---------------- filler machinery ----------------
        filler = deque()   # items: (label_or_None, fn, est_pe_ns)
        done = set()
        est = {"pe": 0.0, "act": 0.0}

        def pop_one():
            label, fn, cost = filler.popleft()
            fn()
            if label is not None:
                done.add(label)
            est["pe"] += cost

        def need(label):
            while label not in done:
                assert filler, f"gate {label} not in filler"
                pop_one()

        def budget_pops():
            while filler and est["pe"] + RESERVE < est["act"]:
                pop_one()

        def ph1_steps(j):
            """Phase-1 QKV projection for query block j as filler items."""
            jsl = slice(j * TQ, (j + 1) * TQ)
            xr_j = xr_cache[j]
            items = []

            def m_group(m):
                cell = {}

                def mk(d):
                    def fn():
                        if d == 0:
                            cell["ps"] = ps_sh.tile(
                                [128, TQ], F32, tag="sh", name=f"psqk{j}_{m}")
                        ps = cell["ps"]
                        last = (d == NDCH - 1) and not qkv_bias
                        nc.tensor.matmul(
                            ps[:], wqk_m[m][:, d * 128:(d + 1) * 128],
                            xr_j[:, d * TQ:(d + 1) * TQ],
                            start=(d == 0), stop=last)
                        if d == NDCH - 1:
                            if qkv_bias:
                                nc.tensor.matmul(
                                    ps[:], bqk_r[0:1, m * 128:(m + 1) * 128],
                                    ones512_r[:], start=False, stop=True)
                            dst = q2[m] if m < 4 else kT[m - 4]
                            nc.vector.tensor_copy(dst[:, jsl], ps[:])
                    return fn

                return ([(None, mk(d), EST_FILL) for d in range(NDCH - 1)]
                        + [(("m", j, m), mk(NDCH - 1), EST_FILL + 60)])

            def v_group(tt):
                c = tt % 4
                cell = {}

                def mk(d):
                    def fn():
                        if d == 0:
                            cell["ps"] = ps_sh.tile(
                                [128, DL], F32, tag="sh", name=f"psv{tt}")
                        ps = cell["ps"]
                        last = (d == NDCH - 1) and not qkv_bias
                        nc.tensor.matmul(
                            ps[:],
                            xr_j[:, d * TQ + c * 128:d * TQ + (c + 1) * 128],
                            wv2[:, d * DL:(d + 1) * DL],
                            start=(d == 0), stop=last)
                        if d == NDCH - 1:
                            if qkv_bias:
                                nc.tensor.matmul(
                                    ps[:], ones_r[:, 0:128], bv_r[:],
                                    start=False, stop=True)
                            src = ps.rearrange("p (h x) -> p h x", h=HL)
                            dst = vs[tt][:, 0:HL * 65].rearrange(
                                "p (h x) -> p h x", x=65)[:, :, 0:64]
                            nc.vector.tensor_copy(dst, src)
                    return fn

                return ([(None, mk(d), EST_FILL) for d in range(NDCH - 1)]
                        + [(("v", j, tt % 4), mk(NDCH - 1), EST_FILL + 60)])

            # pair-i Q/K groups unlock attention pairs in sequence; V groups
            # interleave so forced drains stay small.
            for i in range(4):
                items += m_group(i) + m_group(4 + i)
                items += v_group(4 * j + i)
            return items

        def proj_steps(j):
            items = []
            for t in range(4 * j, 4 * j + 4):
                for nb in range(2):
                    def fn(t=t, nb=nb):
                        nsl = slice(nb * 512, (nb + 1) * 512)
                        ps3 = ps_sh.tile([128, TQ], F32, tag="sh",
                                         name=f"ps3_{t}_{nb}")
                        for k in range(4):
                            nc.tensor.matmul(
                                ps3[:], yT[k][:, t * 128:(t + 1) * 128],
                                wp2[:, k * D + nb * 512:k * D + (nb + 1) * 512],
                                start=(k == 0), stop=(k == 3))
                        ot = opool.tile([128, TQ], BF16, tag="ot",
                                        name=f"ot{t}_{nb}")
                        nc.vector.tensor_copy(ot[:], ps3[:])
                        outq[(t + nb) % 2].dma_start(
                            out_d[t * 128:(t + 1) * 128, nsl], ot[:])
                    items.append((None, fn, 4 * EST_FILL))
            return items

        outq = [nc.gpsimd, nc.scalar]
        pair_no = [0]      # global head-pair counter (norm gating)
        proj_hold = []     # early blocks' projection, deferred to the last
                           # attention block where the PE would otherwise
                           # run out of filler and go cold

        # ---------------- main pipelined loop ----------------
        if causal:
            xload(1)
            filler.extend(ph1_steps(0))
        else:
            for j in range(1, NQB):
                xload(j)
            for j in range(NQB):
                filler.extend(ph1_steps(j))

        for j in range(NQB):
            jsl = slice(j * TQ, (j + 1) * TQ)
            cs = list(range(4 * (j + 1))) if causal else list(range(NKC))
            if causal and j + 2 < NQB:
                xload(j + 2)
            if causal and j + 1 < NQB:
                filler.extend(ph1_steps(j + 1))
            if causal and j == NQB - 1:
                filler.extend(proj_hold)
                proj_hold.clear()

            for i in range(4):          # head pair (2i, 2i+1)
                hA, hB = 2 * i, 2 * i + 1
                need(("m", j, i))
                poA = ps_o.tile([128, TQ], F32, tag="po", name=f"poA{j}_{i}")
                poB = ps_o.tile([128, TQ], F32, tag="po", name=f"poB{j}_{i}")

                pend = deque()  # pipeline: PV(c) emitted after QK(c+2)
                first_pv = [True]

                def pv_pop(stop):
                    pc, ppt = pend.popleft()
                    need(("v", pc // 4, pc % 4))
                    pv_emit(pc, ppt, first_pv[0], stop)
                    first_pv[0] = False
                def qskip(c):
                    # fully-masked leading query columns of a diagonal chunk
                    if causal and c >= 4 * j:
                        return (c - 4 * j) * TKC
                    return 0

                def pv_emit(pc, ppt, start, stop):
                    k0 = qskip(pc)
                    nc.tensor.matmul(
                        poA[:, k0:TQ], vs[pc][:, hA * 65:hA * 65 + 128],
                        ppt[:, k0:TQ], start=start, stop=stop)
                    nc.tensor.matmul(
                        poB[:, k0:TQ], vs[pc][:, hB * 65:hB * 65 + 128],
                        ppt[:, TQ + k0:2 * TQ], start=start, stop=stop)
                    est["pe"] += EST_PV

                for ci, c in enumerate(cs):
                    need(("m", c // 4, 4 + i))
                    csl = slice(c * TKC, (c + 1) * TKC)
                    k0 = qskip(c)
                    ss = ps_s.tile([TKC, 2 * TQ], F32, tag="ss",
                                   name=f"ss{j}_{i}_{c}")
                    nc.tensor.matmul(ss[:, k0:TQ], kT[i][0:64, csl],
                                     q2[i][0:64, j * TQ + k0:(j + 1) * TQ],
                                     start=True, stop=True)
                    nc.tensor.matmul(ss[:, TQ + k0:2 * TQ], kT[i][64:128, csl],
                                     q2[i][64:128, j * TQ + k0:(j + 1) * TQ],
                                     start=True, stop=True)
                    est["pe"] += EST_QK
                    pt = ppool.tile([TKC, 2 * TQ], BF16, tag="pt",
                                    name=f"pt{j}_{i}_{c}")
                    nc.scalar.activation(pt[:], ss[:], EXP, scale=0.125)
                    est["act"] += EST_EXP
                    if causal and c >= 4 * j:
                        # only the 128-wide diagonal band is partially masked
                        bsl = slice(k0, k0 + TKC)
                        nc.vector.tensor_mul(pt[:, bsl], pt[:, bsl], maskb[:])
                        bslB = slice(TQ + k0, TQ + k0 + TKC)
                        nc.vector.tensor_mul(pt[:, bslB], pt[:, bslB],
                                             maskb[:])
                    if len(pend) >= 2:
                        pv_pop(False)
                    pend.append((c, pt))
                    # bound accounting drift so filler keeps flowing through
                    # DMA-paced stretches where the PE is stall-bound anyway
                    if est["pe"] > est["act"] + 3000:
                        est["pe"] = est["act"] + 3000
                    budget_pops()
                while len(pend) > 1:
                    pv_pop(False)
                budget_pops()
                pv_pop(True)

                # immediate DVE part of softmax normalization; the
                # broadcast+multiply is queued as a filler item so the PE
                # stream never pauses at pair/phase boundaries.
                if pair_no[0] >= 2:
                    need(("n", pair_no[0] - 2))   # npool buffer rotation
                rr, osb = [], []
                for h, po in ((hA, poA), (hB, poB)):
                    o_sb = npool.tile([64, TQ], BF16, tag="o_sb",
                                      name=f"ob{j}_{h}")
                    nc.vector.tensor_copy(o_sb[:], po[0:64, :])
                    sums = npool.tile([1, TQ], F32, tag="sums",
                                      name=f"sm{j}_{h}")
                    nc.vector.tensor_copy(sums[:], po[64:65, :])
                    recip = npool.tile([1, TQ], F32, tag="recip",
                                       name=f"rc{j}_{h}")
                    nc.vector.reciprocal_approx_fast(
                        out=recip[:], in_=sums[:])
                    rr.append(recip)
                    osb.append(o_sb)

                def norm_fn(i=i, jsl=jsl, osb=osb, rr=rr, j=j, hA=hA):
                    for hp in (0, 1):
                        pb = npool.tile([64, TQ], F32, tag="pb",
                                        name=f"pb{j}_{hA + hp}")
                        nc.gpsimd.partition_broadcast(pb[:], rr[hp][:])
                        nc.vector.tensor_mul(
                            yT[i][hp * 64:(hp + 1) * 64, jsl],
                            osb[hp][:], pb[:])
                filler.append((("n", pair_no[0]), norm_fn, 50))
                pair_no[0] += 1

            if causal and j == NQB - 1:
                break                  # final block projection done below
            if causal and j < 2:
                proj_hold.extend(proj_steps(j))
            else:
                filler.extend(proj_steps(j))

        if causal:
            while filler:              # drain ph1/proj/norm backlog
                pop_one()
            # final-block projection, pair-major: MMs for pair k only wait
            # norm(k), so the PE streams through the tail without stalls.
            jf = NQB - 1
            s1 = ps_s.tile([TKC, 2 * TQ], F32, tag="ss", name="pf_a")
            s2 = ps_s.tile([TKC, 2 * TQ], F32, tag="ss", name="pf_b")
            s3 = ps_sh.tile([128, TQ], F32, tag="sh", name="pf_c")
            s4 = ps_sh.tile([128, TQ], F32, tag="sh", name="pf_d")
            s5 = ps_o.tile([128, TQ], F32, tag="po", name="pf_e")
            s6 = ps_o.tile([128, TQ], F32, tag="po", name="pf_f")
            slots = [s1[:, 0:TQ], s1[:, TQ:2 * TQ], s2[:, 0:TQ],
                     s2[:, TQ:2 * TQ], s3[:], s4[:], s5[:], s6[:]]
            steps = [(t, nb) for t in range(4 * jf, 4 * jf + 4)
                     for nb in range(2)]
            for k in range(4):
                for idx, (t, nb) in enumerate(steps):
                    nsl = slice(nb * 512, (nb + 1) * 512)
                    nc.tensor.matmul(
                        slots[idx], yT[k][:, t * 128:(t + 1) * 128],
                        wp2[:, k * D + nb * 512:k * D + (nb + 1) * 512],
                        start=(k == 0), stop=(k == 3))
                    if k == 3:
                        ot = opool.tile([128, TQ], BF16, tag="ot",
                                        name=f"otf{idx}")
                        nc.vector.tensor_copy(ot[:], slots[idx])
                        outq[idx % 2].dma_start(
                            out_d[t * 128:(t + 1) * 128, nsl], ot[:])

        while filler:                  # flush remaining projection work
            pop_one()

    nc.compile()
    return nc


def _get_nc(causal: bool, qkv_bias: bool = False):
    key = (causal, qkv_bias)
    if key not in _CACHE:
        _CACHE[key] = _build(causal, qkv_bias)
    return _CACHE[key]


def _host_masks() -> np.ndarray:
    i = np.arange(TKC)[:, None]
    jj = np.arange(TKC)[None, :]
    return np.ascontiguousarray(
        (jj >= i).astype(np.float32).astype(ml_dtypes.bfloat16))


def _make_in_maps(x, W_qkv, b_qkv, W_proj):
    masks_np = _host_masks()
    in_maps = []
    for core in range(N_CORES):
        b, g = core // 2, core % 2
        qc = slice(g * DL, (g + 1) * DL)
        kc = slice(D + g * DL, D + (g + 1) * DL)
        vc = slice(2 * D + g * DL, 2 * D + (g + 1) * DL)
        bf = ml_dtypes.bfloat16
        wqk_full = np.concatenate([W_qkv[:, qc], W_qkv[:, kc]], axis=1)
        # [D, 2DL] -> [m, p, d, m2] so each m-chunk is one contiguous DMA
        wqk_t = wqk_full.reshape(8, 128, 8, 128).transpose(2, 1, 0, 3)
        in_maps.append({
            "xT": np.ascontiguousarray(
                x[b].T.reshape(NDCH, 128, NQB, TQ).transpose(2, 1, 0, 3)
                .reshape(NQB, 128, NDCH * TQ).astype(bf)),
            "wqk": np.ascontiguousarray(
                wqk_t.reshape(8, 128, 2 * DL).astype(bf)),
            "wv": np.ascontiguousarray(
                W_qkv[:, vc].reshape(NDCH, 128, DL).transpose(1, 0, 2)
                .reshape(128, NDCH * DL).astype(bf)),
            "bqk": np.ascontiguousarray(
                np.concatenate([b_qkv[qc], b_qkv[kc]]).reshape(1, 2 * DL)),
            "bv": np.ascontiguousarray(b_qkv[vc].reshape(1, DL)),
            "wproj": np.ascontiguousarray(
                W_proj[g * DL:(g + 1) * DL, :].reshape(4, 128, D)
                .transpose(1, 0, 2).reshape(128, 4 * D).astype(bf)),
            "masks": masks_np,
        })
    return in_maps


def kernel(x, mask, W_qkv, b_qkv, W_proj, b_proj):
    x = np.asarray(x, dtype=np.float32)
    mask2d = np.asarray(mask, dtype=np.int32).reshape(T, T)
    W_qkv = np.asarray(W_qkv, dtype=np.float32)
    b_qkv = np.asarray(b_qkv, dtype=np.float32)
    W_proj = np.asarray(W_proj, dtype=np.float32)
    b_proj = np.asarray(b_proj, dtype=np.float32)

    if np.array_equal(mask2d, np.tril(np.ones((T, T), dtype=np.int32))):
        causal = True
    elif np.all(mask2d == 1):
        causal = False
    else:
        raise NotImplementedError("only causal (tril) or all-ones masks")

    qkv_bias = bool(np.any(b_qkv != 0.0))
    nc = _get_nc(causal, qkv_bias)
    in_maps = _make_in_maps(x, W_qkv, b_qkv, W_proj)
    res = run_bass_kernel_spmd(nc, in_maps, core_ids=list(range(N_CORES)))
    out = np.empty((B, T, D), dtype=np.float32)
    for b in range(B):
        out[b] = (res.results[2 * b]["out"].astype(np.float32)
                  + res.results[2 * b + 1]["out"].astype(np.float32)
                  + b_proj[None, :])
    return out
